# revision 17
# baseline (speedup 1.0000x reference)
"""KMeans clustering step (vq_codebook) on 8 trn2 NeuronCores.

Data-parallel over N = B*S = 131072 flattened points (D=256, K=512 clusters).
Each core handles 16384 points in 128 tiles of 128 points:
  1. DMA x tile [128, 256+pad] (ones col appended for counts)
  2. PE-transpose x tile -> xT [256d, 128pts] (PSUM), ACT copy -> SBUF
  3. s = x @ (-2 C^T) + c2  via 3 fp32r matmuls into PSUM [128, 512]
     (x2 omitted: constant per row, does not change the argmin)
  4. DVE row-min + is_equal -> one-hot [128, 512] f32 in SBUF
  5. 4 fp32r matmuls accumulate sums[cl,257] += onehot_chunk^T @ [x | 1]
  6. DMA one-hot tile to HBM
Partial [512, 257] sums|counts per core are reduced on host (tiny),
followed by the EMA update.
"""

import sys
from contextlib import ExitStack

import numpy as np

sys.path.insert(0, "/opt/trn_rl_repo")

P = 128
D = 256
K = 512
N_CORES = 8
N_TOTAL = 128 * 1024          # B*S
N_CORE = N_TOTAL // N_CORES   # 16384 points per core
T = N_CORE // P               # 128 tiles per core
XCOLS = D + 4                 # x tile cols: 256 data + ones col + pad
CCOLS = 2 * K + P + K + P     # packed consts cols
CHUNK = 8                     # tiles per x preload slab (~1 MB DMAs)
OHG = 4                       # tiles per one-hot store group (1 MB stores)

_CACHE = {}


def _build():
    import concourse.bass as bass
    import concourse.tile as tile
    from concourse import bacc, mybir

    f32 = mybir.dt.float32
    f32r = mybir.dt.float32r
    nc = bacc.Bacc(trn_type="TRN2")

    x_in = nc.dram_tensor("x", [N_CORE, D + 1], f32, kind="ExternalInput")
    # packed constants, one DMA: [:,0:512]=-2C^T rows 0:128, [:,512:1024]=-2C^T
    # rows 128:256, [:,1024:1152]=identity, row0 [1152:1664]=||c||^2,
    # row0 [1664:1792]=ones
    co_in = nc.dram_tensor("consts", [P, CCOLS], f32, kind="ExternalInput")
    oh_out = nc.dram_tensor("onehot", [N_CORE, K], f32, kind="ExternalOutput")
    ps_out = nc.dram_tensor("partials", [K, D + 1], f32, kind="ExternalOutput")

    with ExitStack() as ctx:
        tc = ctx.enter_context(tile.TileContext(nc))
        cpool = ctx.enter_context(tc.tile_pool(name="const", bufs=1))
        xpool = ctx.enter_context(tc.tile_pool(name="xp", bufs=1))
        xtpool = ctx.enter_context(tc.tile_pool(name="xtp", bufs=3))
        ohpool = ctx.enter_context(tc.tile_pool(name="ohp", bufs=2))
        mpool = ctx.enter_context(tc.tile_pool(name="mp", bufs=4))
        pdist = ctx.enter_context(tc.tile_pool(name="pdist", bufs=2, space="PSUM"))
        pxt = ctx.enter_context(tc.tile_pool(name="pxt", bufs=2, space="PSUM"))
        psums = ctx.enter_context(tc.tile_pool(name="psums", bufs=1, space="PSUM"))
        ocopy = ctx.enter_context(tc.tile_pool(name="ocp", bufs=2))

        # packed constants, single DMA -> single semaphore lane for PE waits
        co = cpool.tile([P, CCOLS], f32, tag="consts")
        nc.sync.dma_start(co[:, :], co_in[:, :])
        cts = [co[:, 0:K], co[:, K:2 * K]]
        ident = co[:, 2 * K:2 * K + P]
        c2 = co[0:1, 2 * K + P:2 * K + P + K]
        ones = co[0:1, 2 * K + P + K:2 * K + P + K + P]

        sums_ps = [psums.tile([P, D + 1], f32, tag=f"s{j}", name=f"s{j}")
                   for j in range(4)]

        # preload all of x in CHUNK-tile slabs: big DMAs, no buffer cycling,
        # so every DMA carries at most one semaphore wait
        W = D + 1
        xslabs = [xpool.tile([P, CHUNK, W], f32, tag=f"x{b}", name=f"x{b}")
                  for b in range(T // CHUNK)]
        for b in range(T // CHUNK):
            src = x_in[b * CHUNK * P:(b + 1) * CHUNK * P, :]
            nc.sync.dma_start(xslabs[b][:, :, :],
                              src.rearrange("(i p) c -> p i c", p=P))

        ohg = None
        for i in range(T):
            xe = xslabs[i // CHUNK][:, i % CHUNK, :]

            # s = x @ (-2 C^T) + c2   [128 pts, 512 cl] in PSUM
            # c2 rank-1 matmul first: reads only consts -> no new PE waits
            s_ps = pdist.tile([P, K], f32)
            nc.tensor.matmul(s_ps[:, :], ones, c2, start=True, stop=False)

            # transpose x tile on PE: [128 pts, 256 d] -> [256 d, 128 pts]
            xt_ps = pxt.tile([P, D], f32)
            nc.tensor.transpose(xt_ps[:, 0:P], xe[:, 0:P], ident)
            nc.tensor.transpose(xt_ps[:, P:D], xe[:, P:D], ident)
            xt = xtpool.tile([P, D], f32)
            nc.scalar.copy(xt[:, :], xt_ps[:, :])

            nc.tensor.matmul(s_ps[:, :], xt[:, 0:P],
                             cts[0], start=False, stop=False)
            nc.tensor.matmul(s_ps[:, :], xt[:, P:D],
                             cts[1], start=False, stop=True)

            # argmin -> one-hot
            m = mpool.tile([P, 1], f32)
            nc.vector.tensor_reduce(m[:, :], s_ps[:, :], axis=mybir.AxisListType.X,
                                    op=mybir.AluOpType.min)
            if i % OHG == 0:
                ohg = ohpool.tile([P, OHG, K], f32, tag="ohg", name="ohg")
            oh = ohg[:, i % OHG, :]
            nc.vector.tensor_scalar(oh, s_ps[:, :], m[:, :], None,
                                    op0=mybir.AluOpType.is_equal)

            # sums[cl, 0:256] += onehot^T @ x ; sums[cl, 256] += counts
            for j in range(4):
                nc.tensor.matmul(sums_ps[j][:, :],
                                 oh[:, j * P:(j + 1) * P],
                                 xe[:, 0:D + 1],
                                 start=(i == 0), stop=(i == T - 1))

            if i % OHG == OHG - 1:
                g0 = (i - OHG + 1) * P
                dst = oh_out[g0:g0 + OHG * P, :]
                nc.sync.dma_start(dst.rearrange("(i p) k -> p i k", p=P),
                                  ohg[:, :, :])

        for j in range(4):
            sc = ocopy.tile([P, D + 1], f32)
            nc.scalar.copy(sc[:, :], sums_ps[j][:, :])
            nc.sync.dma_start(ps_out[j * P:(j + 1) * P, :], sc[:, :])

    nc.compile()
    return nc


def kernel(inputs: np.ndarray, centroids: np.ndarray):
    from concourse.bass_utils import run_bass_kernel_spmd

    if "nc" not in _CACHE:
        _CACHE["nc"] = _build()
    nc = _CACHE["nc"]

    x = inputs.reshape(-1, D).astype(np.float32)
    x = np.ascontiguousarray(
        np.concatenate([x, np.ones((x.shape[0], 1), np.float32)], axis=1))
    c = np.ascontiguousarray(centroids.astype(np.float32))
    cts = -2.0 * c.T                                         # [D, K]
    c2 = (c * c).sum(axis=1, dtype=np.float32)               # [K]

    co = np.zeros((P, CCOLS), np.float32)
    co[:, 0:K] = cts[0:P, :]
    co[:, K:2 * K] = cts[P:D, :]
    co[:, 2 * K:2 * K + P] = np.eye(P, dtype=np.float32)
    co[0, 2 * K + P:2 * K + P + K] = c2
    co[0, 2 * K + P + K:2 * K + P + K + P] = 1.0

    in_maps = []
    for core in range(N_CORES):
        shard = np.ascontiguousarray(x[core * N_CORE:(core + 1) * N_CORE])
        in_maps.append({"x": shard, "consts": co})

    import time as _time
    t0 = _time.perf_counter()
    res = run_bass_kernel_spmd(nc, in_maps, core_ids=list(range(N_CORES)))
    t1 = _time.perf_counter()
    _CACHE["exec_time_ns"] = int((t1 - t0) * 1e9)
    if res.exec_time_ns is not None:
        _CACHE["exec_time_ns"] = res.exec_time_ns
    outs = res.results

    clusters = np.concatenate([outs[c]["onehot"] for c in range(N_CORES)], axis=0)
    clusters = clusters.reshape(inputs.shape[0], inputs.shape[1], K)

    part = np.stack([outs[c]["partials"] for c in range(N_CORES)]).sum(axis=0)
    sums = part[:, :D]
    counts = part[:, D]
    new_centroids = sums / counts[:, None]
    updated = 0.9 * c + 0.1 * new_centroids
    return clusters, updated.astype(np.float32)
